# revision 11
# baseline (speedup 1.0000x reference)
"""BioSSMMixer distributed Trainium2 kernel (8 NeuronCores).

Sharding: channel dimension D is split across the 8 cores (the SSM scan is
diagonal in D, so each core scans its own 128 channels with no cross-core
state). The final W_out projection contracts the full D, so the gate tensor
g = y_sp * silu(z) is exchanged with an AllToAll (d-shards -> t-shards) and
each core computes the output rows for its own T/8 slice. The AllToAll and
output matmul for batch row b=0 overlap the b=1 scan.

Host-side prep (not part of HW exec time): W_xd = W_xz[:, :D] @ W_dt is
folded so dt can be computed directly from h (each core only ever needs its
own 128 output channels of x/z/dt); h is pre-transposed to [D, B*T] bf16 so
the contraction axis lands on SBUF partitions without an on-device transpose.

Device layout: all per-channel tensors live as [d=128 partitions, (b,t) free]
tiles. Per (b, n) the recurrence s_t = decay_t*s_{t-1} + inp_t runs as a
single tensor_tensor_scan over the t axis. Bm/Cm rows (which vary with t but
not d) are broadcast across partitions with a one-hot-selector PE matmul.
The y accumulation over n runs on the otherwise-idle GpSimd engine.
"""

import os
import numpy as np
import ml_dtypes

B, T, D, N = 2, 1024, 1024, 16
NCORES = 8
DL = D // NCORES        # 128 channels per core
TL = T // NCORES        # 128 timesteps per core (output slice)
R = B * T               # 2048 rows, b-major: row = b*T + t
KT = D // 128           # 8 contraction tiles
H = 512                 # psum half-tile

BF16 = ml_dtypes.bfloat16

# Filled by kernel() when KERNEL_TRACE=1: exec_time_ns, trace tmpdir.
LAST = {}

_GRAPH_CACHE = {}


def _patch_act_tables():
    """Order activation tables so Exp and Ln resolve to the combined
    natural_log_exp_and_others table (otherwise the table-load pass
    ping-pongs between exp_and_others and natural_log)."""
    import concourse.hw_specs as hw_specs
    import concourse.bacc as bacc_mod
    orig = hw_specs.get_activation_tables.__wrapped__
    import functools

    @functools.cache
    def reordered(arch):
        # Table index (act_func_set_id) must stay aligned with
        # act_info.json's order — never reorder. Prune Exp/Ln from the
        # single-function tables instead so the load pass resolves both
        # to natural_log_exp_and_others.
        import concourse.mybir as mybir
        Act = mybir.ActivationFunctionType
        t = {k: set(v) for k, v in orig(arch).items()}
        if "natural_log_exp_and_others" in t:
            for k in ("exp_and_others", "exp_and_friends"):
                t.get(k, set()).discard(Act.Exp)
            t.get("natural_log", set()).discard(Act.Ln)
        return t

    hw_specs.get_activation_tables = reordered
    bacc_mod.get_activation_tables = reordered


def _build_graph():
    if "nc" in _GRAPH_CACHE:
        return _GRAPH_CACHE["nc"]

    import concourse.bacc as bacc
    import concourse.mybir as mybir
    from concourse import tile

    if os.environ.get('ACT_PATCH','1')=='1':
        _patch_act_tables()

    f32 = mybir.dt.float32
    bf16 = mybir.dt.bfloat16
    Alu = mybir.AluOpType
    Act = mybir.ActivationFunctionType

    nc = bacc.Bacc(
        "TRN2",
        target_bir_lowering=False,
        debug=False,
        enable_asserts=True,
        num_devices=NCORES,
    )

    hT_d = nc.dram_tensor("hT", [B, KT, 128, T], bf16, kind="ExternalInput")
    wpack_d = nc.dram_tensor("wpack", [D, 3 * DL + 2 * N], bf16,
                             kind="ExternalInput")
    wout_d = nc.dram_tensor("wout", [D, D], bf16, kind="ExternalInput")
    acol_d = nc.dram_tensor("acol", [DL, N], f32, kind="ExternalInput")
    bdt_d = nc.dram_tensor("bdt", [DL, 1], f32, kind="ExternalInput")
    dsk_d = nc.dram_tensor("dsk", [DL, 1], f32, kind="ExternalInput")
    nvth_d = nc.dram_tensor("nvth", [DL, 1], f32, kind="ExternalInput")
    hres_d = nc.dram_tensor("hres", [B, TL, D], f32, kind="ExternalInput")
    sel_d = nc.dram_tensor("sel", [2 * N, 2 * N * 128], bf16,
                           kind="ExternalInput")
    out_d = nc.dram_tensor("out", [B, TL, D], f32, kind="ExternalOutput")

    with tile.TileContext(nc) as tc:
        with (
            tc.tile_pool(name="const", bufs=1) as cpool,
            tc.tile_pool(name="work", bufs=1) as wpool,
            tc.tile_pool(name="sc", bufs=4) as scpool,
            tc.tile_pool(name="px", bufs=2, space="PSUM") as pxpool,
            tc.tile_pool(name="dram", bufs=1, space="DRAM") as dpool,
        ):
            # ---- constant loads -------------------------------------------
            # Issue order matters: each queue delivers FIFO, so weights go
            # first (PE's first matmul gates on them), then hT b=0 tiles,
            # then hT b=1 and everything the out stage needs.
            hT = cpool.tile([128, KT, R], bf16)
            _eng = [nc.sync, nc.scalar, nc.gpsimd]
            WP = 3 * DL + 2 * N
            wpk = cpool.tile([128, KT, WP], bf16)
            acol = cpool.tile([DL, N], f32)
            bdt = cpool.tile([DL, 1], f32)
            dsk = cpool.tile([DL, 1], f32)
            nvth = cpool.tile([DL, 1], f32)
            sel = cpool.tile([2 * N, 2 * N * 128], bf16)
            wout = cpool.tile([128, KT, D], bf16)
            hres0 = cpool.tile([TL, D], f32)
            hres1 = cpool.tile([TL, D], f32)
            nc.gpsimd.dma_start(bdt[:], bdt_d[:])
            nc.gpsimd.dma_start(acol[:], acol_d[:])
            nc.gpsimd.dma_start(dsk[:], dsk_d[:])
            nc.gpsimd.dma_start(nvth[:], nvth_d[:])
            # warmup collective: absorbs the one-time CC route setup and the
            # SPMD launch skew (~30-45us) under the input ramp, so the real
            # AllToAlls later run at warm speed (~10us instead of ~40us)
            warm_in = dpool.tile([NCORES, 1], f32, tag="warmi")
            warm_out = dpool.tile([NCORES, 1], f32, tag="warmo")
            nc.sync.dma_start(warm_in[:], bdt[0:NCORES, :])
            nc.gpsimd.collective_compute(
                "AllToAll", Alu.bypass,
                replica_groups=[list(range(NCORES))],
                ins=[warm_in[:].opt()], outs=[warm_out[:].opt()])
            # warm the gpsimd partition_broadcast ISA library (first use
            # pays a ~10us load); dummy broadcast of bdt into scratch
            pbwarm = cpool.tile([128, 1], f32)
            nc.gpsimd.partition_broadcast(pbwarm[:], bdt[0:1, 0:1])
            for j in range(KT):
                _eng[j % 3].dma_start(wpk[:, j, :],
                                      wpack_d[j * 128:(j + 1) * 128, :])
            for half in range(B):
                ts_h = slice(half * T, (half + 1) * T)
                for j in range(KT):
                    _eng[j % 3].dma_start(hT[:, j, ts_h], hT_d[half, j])
                if half == 0:
                    nc.sync.dma_start(sel[:], sel_d[:])
            # out-stage tensors ride the scalar HW queue after the hT ramp;
            # they land by ~60us, well before the out matmuls need them
            nc.scalar.dma_start(hres0[:], hres_d[0])
            nc.scalar.dma_start(hres1[:], hres_d[1])
            for j in range(KT):
                nc.scalar.dma_start(wout[:, j, :],
                                    wout_d[j * 128:(j + 1) * 128, :])

            # ---- projections: xT/zT/dtT [128 d, R], BmCm [32, R] ----------
            xT = wpool.tile([128, R], bf16)
            dtT = wpool.tile([128, R], bf16)
            zT = wpool.tile([128, R], bf16)
            dtx = wpool.tile([128, R], bf16)
            yT = wpool.tile([128, R], bf16)
            yT2 = wpool.tile([128, T], bf16)
            bmcm = wpool.tile([2 * N, R], bf16)
            # Bm rows collapsed to partition 0 so partition_broadcast (which
            # can only read partition 0) can fan out row n per (b, n).
            # One b at a time: b=1's collapse waits for b=0's last broadcast
            bmflat = wpool.tile([1, N * T], bf16)
            gT = wpool.tile([128, R], bf16)

            for bb in range(B):
                cs = slice(bb * T, (bb + 1) * T)
                pd = pxpool.tile([128, T], f32, tag="pb", bufs=3)
                for hh in range(2):
                    hs = slice(bb * T + hh * H, bb * T + (hh + 1) * H)
                    for j in range(KT):
                        nc.tensor.matmul(pd[:, hh * H:(hh + 1) * H],
                                         wpk[:, j, 2 * DL:3 * DL], hT[:, j, hs],
                                         start=(j == 0), stop=(j == KT - 1))
                # softplus(x+b) = ln(1 + exp(x+b)); Exp and Ln share a table
                et = scpool.tile([128, T], bf16, tag="et", bufs=1)
                nc.scalar.activation(et[:], pd[:], Act.Exp, bias=bdt[:, 0:1])
                nc.scalar.activation(dtT[:, cs], et[:], Act.Ln, bias=1.0)
                px = pxpool.tile([128, T], f32, tag="pb", bufs=3)
                for hh in range(2):
                    hs = slice(bb * T + hh * H, bb * T + (hh + 1) * H)
                    for j in range(KT):
                        nc.tensor.matmul(px[:, hh * H:(hh + 1) * H],
                                         wpk[:, j, 0:DL], hT[:, j, hs],
                                         start=(j == 0), stop=(j == KT - 1))
                nc.vector.tensor_copy(xT[:, cs], px[:])
                pm = pxpool.tile([32, T], f32, tag="pb", bufs=3)
                for hh in range(2):
                    hs = slice(bb * T + hh * H, bb * T + (hh + 1) * H)
                    for j in range(KT):
                        nc.tensor.matmul(pm[:, hh * H:(hh + 1) * H],
                                         wpk[:, j, 3 * DL:WP], hT[:, j, hs],
                                         start=(j == 0), stop=(j == KT - 1))
                nc.vector.tensor_copy(bmcm[:, cs], pm[:])
                if bb == 0:
                    nc.sync.dma_start(bmflat[0:1, :], bmcm[0:N, cs])
                nc.vector.tensor_mul(dtx[:, cs], dtT[:, cs], xT[:, cs])

            # ---- per-b: scan over (n), epilogue, AllToAll, out matmul -----
            gT_r = gT[:].rearrange("p (b t) -> p b t", b=B)

            def z_proj(b):
                # z projection is only needed by b's epilogue; emitted late
                # so PE runs it after the time-critical broadcasts
                pz = pxpool.tile([128, T], f32, tag="pb", bufs=3)
                for hh in range(2):
                    hs = slice(b * T + hh * H, b * T + (hh + 1) * H)
                    for j in range(KT):
                        nc.tensor.matmul(pz[:, hh * H:(hh + 1) * H],
                                         wpk[:, j, DL:2 * DL], hT[:, j, hs],
                                         start=(j == 0), stop=(j == KT - 1))
                nc.vector.tensor_copy(zT[:, b * T:(b + 1) * T], pz[:])

            def n_loop(b):
                bs = slice(b * T, (b + 1) * T)
                if b == 1:
                    nc.sync.dma_start(bmflat[0:1, :], bmcm[0:N, bs])
                for n in range(N):
                    decay = scpool.tile([128, T], bf16, tag="decay", bufs=3)
                    nc.scalar.activation(decay[:], dtT[:, bs], Act.Exp,
                                         scale=acol[:, n:n + 1])
                    # Bm broadcast: gpsimd partition_broadcast reads bmcm row
                    # n straight from SBUF and fans it to 128 partitions as
                    # bf16 — no PE matmul, no PSUM, and the inp multiply
                    # below runs in DVE 2x mode (all-bf16 operands)
                    bmb = scpool.tile([128, T], bf16, tag="bmb", bufs=3)
                    nc.gpsimd.partition_broadcast(
                        bmb[:], bmflat[0:1, n * T:(n + 1) * T])
                    inp = scpool.tile([128, T], bf16, tag="inp", bufs=3)
                    nc.vector.tensor_mul(inp[:], dtx[:, bs], bmb[:])
                    s = scpool.tile([128, T], bf16, tag="s", bufs=3)
                    nc.vector.tensor_tensor_scan(s[:], decay[:], inp[:], 0.0,
                                                 Alu.mult, Alu.add)
                    # Cm broadcast: PE selector matmul into PSUM, then the
                    # scalar engine copies it out as bf16 so the tmp multiply
                    # also runs at DVE 2x
                    pcm = pxpool.tile([128, T], f32, tag="pb", bufs=3)
                    for hh in range(2):
                        hs_d = slice(hh * H, (hh + 1) * H)
                        hs_s = slice(b * T + hh * H, b * T + (hh + 1) * H)
                        nc.tensor.matmul(pcm[:, hs_d],
                                         sel[:, (N + n) * 128:(N + n + 1) * 128],
                                         bmcm[:, hs_s], start=True, stop=True)
                    cmb = scpool.tile([128, T], bf16, tag="cmb", bufs=3)
                    nc.scalar.activation(cmb[:], pcm[:], Act.Copy)
                    tmp = scpool.tile([128, T], bf16, tag="tmp", bufs=2)
                    nc.vector.tensor_mul(tmp[:], s[:], cmb[:])
                    # two parallel accumulation chains halve the serial
                    # latency of the y-reduction; the last four adds run on
                    # DVE (0.7us each) to shorten the chain into the epilogue
                    acc = yT[:, bs] if n % 2 == 0 else yT2[:]
                    if n < 2:
                        nc.gpsimd.tensor_copy(acc, tmp[:])
                    elif n < N - 4:
                        nc.gpsimd.tensor_add(acc, acc, tmp[:])
                    else:
                        nc.vector.tensor_add(acc, acc, tmp[:])
                    if b == 1 and n == 1:
                        z_proj(1)
                nc.vector.tensor_add(yT[:, bs], yT[:, bs], yT2[:])

            def epilogue(b):
                # y += D_skip*x ; spike = sigmoid(10y - 10vth) ; g = y*spk*silu(z)
                # y-independent part first: silu(z) overlaps the y chain
                bs = slice(b * T, (b + 1) * T)
                sgz = scpool.tile([128, T], bf16, tag="sgz", bufs=1)
                nc.scalar.activation(sgz[:], zT[:, bs], Act.Sigmoid)
                tz = scpool.tile([128, T], bf16, tag="t2", bufs=1)
                nc.vector.tensor_mul(tz[:], sgz[:], zT[:, bs])
                nc.vector.scalar_tensor_tensor(yT[:, bs], xT[:, bs],
                                               dsk[:, 0:1], yT[:, bs],
                                               Alu.mult, Alu.add)
                spk = scpool.tile([128, T], bf16, tag="spk", bufs=1)
                nc.scalar.activation(spk[:], yT[:, bs], Act.Sigmoid,
                                     scale=10.0, bias=nvth[:, 0:1])
                t1 = scpool.tile([128, T], bf16, tag="t1", bufs=1)
                nc.vector.tensor_mul(t1[:], spk[:], tz[:])
                nc.vector.tensor_mul(gT[:, bs], t1[:], yT[:, bs])

            a2a_out_t = [None, None]

            def a2a(b):
                # AllToAll this b's g: d-shards -> t-shards
                a2a_in = dpool.tile([NCORES, DL, TL], bf16, tag=f"a2ai{b}")
                a2a_out = dpool.tile([NCORES, DL, TL], bf16, tag=f"a2ao{b}")
                nc.sync.dma_start(
                    a2a_in[:].rearrange("j p t -> p j t"),
                    gT_r[:, b, :].rearrange("p (j t) -> p j t", j=NCORES))
                nc.gpsimd.collective_compute(
                    "AllToAll",
                    Alu.bypass,
                    replica_groups=[list(range(NCORES))],
                    ins=[a2a_in[:].opt()],
                    outs=[a2a_out[:].opt()],
                )
                a2a_out_t[b] = a2a_out

            def ga_load(b):
                ga = wpool.tile([128, NCORES, TL], bf16, tag=f"ga{b}")
                nc.sync.dma_start(ga[:],
                                  a2a_out_t[b][:].rearrange("j p t -> p j t"))
                return ga

            def out_mm(b, ga):
                # out rows for this b: g_full @ W_out (residual sub deferred)
                osb = wpool.tile([TL, D], f32, tag=f"osb{b}")
                pos = []
                for eh in range(2):
                    es = slice(eh * H, (eh + 1) * H)
                    po = pxpool.tile([128, H], f32, tag="px")
                    for j in range(NCORES):
                        nc.tensor.matmul(po[:], ga[:, j, :], wout[:, j, es],
                                         start=(j == 0), stop=(j == NCORES - 1))
                    pos.append(po)
                return osb, pos

            def out_sub_store(b, osb, pos):
                hres_t = hres0 if b == 0 else hres1
                for eh in range(2):
                    es = slice(eh * H, (eh + 1) * H)
                    nc.vector.tensor_sub(osb[:, es], pos[eh][:], hres_t[:, es])
                    nc.sync.dma_start(out_d[b][:, es], osb[:, es])

            # b=0's out matmuls are deferred past b=1's n-loop so the
            # in-order PE queue never head-of-line blocks on the collective
            n_loop(0)
            z_proj(0)
            epilogue(0)
            a2a(0)
            n_loop(1)           # overlaps b=0's AllToAll
            ga0 = ga_load(0)
            osb0, pos0 = out_mm(0, ga0)
            epilogue(1)
            out_sub_store(0, osb0, pos0)
            a2a(1)
            ga1 = ga_load(1)
            osb1, pos1 = out_mm(1, ga1)
            out_sub_store(1, osb1, pos1)

    nc.compile()
    _GRAPH_CACHE["nc"] = nc
    return nc


def _install_ntff_hook_shim():
    """This image's antenv package lacks axon_hooks; recreate it with the
    ctypes NTFF hook from trn_agent_boot so trace=True yields exec_time_ns."""
    import sys
    import types
    try:
        import antenv.axon_hooks  # noqa: F401
        return
    except ImportError:
        pass
    import antenv
    mod = types.ModuleType("antenv.axon_hooks")
    _h = {"v": None}
    mod.set_axon_ntff_profile_hook = lambda hook: _h.update(v=hook)
    mod.get_axon_ntff_profile_hook = lambda: _h["v"]
    sys.modules["antenv.axon_hooks"] = mod
    antenv.axon_hooks = mod
    try:
        from trn_agent_boot.trn_boot import _ntff_profile_via_ctypes
        hook = _ntff_profile_via_ctypes("/opt/axon/libaxon_pjrt.so")
        mod.set_axon_ntff_profile_hook(hook)
    except Exception as e:  # degrade to no-trace
        print(f"ntff hook shim failed: {e}")


def kernel(hidden_states, W_xz, W_dt, b_dt, A_log, W_B, W_C, D_skip, W_out,
           v_th):
    h = np.asarray(hidden_states, np.float32)
    Wxz = np.asarray(W_xz, np.float32)
    Wdt = np.asarray(W_dt, np.float32)
    bdt = np.asarray(b_dt, np.float32)
    Alog = np.asarray(A_log, np.float32)
    WB = np.asarray(W_B, np.float32)
    WC = np.asarray(W_C, np.float32)
    Dsk = np.asarray(D_skip, np.float32)
    Wout = np.asarray(W_out, np.float32)
    vth = np.asarray(v_th, np.float32)

    # [B, KT, 128, T] so each per-tile DMA reads one contiguous 256KB block
    hT = np.ascontiguousarray(
        h.transpose(2, 0, 1).reshape(KT, 128, B, T).transpose(2, 0, 1, 3)
    ).astype(BF16)
    Wxd = (Wxz[:, :D].astype(np.float64) @ Wdt.astype(np.float64)).astype(
        np.float32)
    A = -np.exp(Alog)
    wbc = np.concatenate([WB, WC], axis=1)
    wout_bf = Wout.astype(BF16)
    sel_np = np.zeros((2 * N, 2 * N * 128), dtype=BF16)
    for n in range(2 * N):
        sel_np[n, n * 128:(n + 1) * 128] = 1.0

    in_maps = []
    for k in range(NCORES):
        ds = slice(k * DL, (k + 1) * DL)
        ts = slice(k * TL, (k + 1) * TL)
        in_maps.append({
            "hT": hT,
            "wpack": np.ascontiguousarray(np.concatenate(
                [Wxz[:, :D][:, ds], Wxz[:, D:][:, ds], Wxd[:, ds], wbc],
                axis=1)).astype(BF16),
            "wout": wout_bf,
            "acol": np.ascontiguousarray(A[ds, :]),
            "bdt": np.ascontiguousarray(bdt[ds].reshape(DL, 1)),
            "dsk": np.ascontiguousarray(Dsk[ds].reshape(DL, 1)),
            "nvth": np.ascontiguousarray(
                (-10.0 * np.maximum(vth[ds], 0.1)).reshape(DL, 1)),
            "hres": np.ascontiguousarray(h[:, ts, :]),
            "sel": sel_np,
        })

    from concourse.bass_utils import run_bass_kernel_spmd

    nc = _build_graph()
    trace = os.environ.get("KERNEL_TRACE", "0") == "1"
    kwargs = {}
    if trace:
        _install_ntff_hook_shim()
        import tempfile
        tmpdir = tempfile.mkdtemp(prefix="biossm_trace_")
        kwargs = dict(trace=True, tmpdir=tmpdir)
        LAST["trace_dir"] = tmpdir
    try:
        res = run_bass_kernel_spmd(nc, in_maps, core_ids=list(range(NCORES)),
                                   **kwargs)
    except Exception:
        # one retry: a crashed prior run can leave sticky device state that
        # clears on the next attempt
        res = run_bass_kernel_spmd(nc, in_maps, core_ids=list(range(NCORES)),
                                   **kwargs)
    LAST["exec_time_ns"] = getattr(res, "exec_time_ns", None)
    out = np.concatenate(
        [np.asarray(res.results[i]["out"], np.float32) for i in range(NCORES)],
        axis=1)
    return out



# revision 12
# speedup vs baseline: 1.6811x; 1.6811x over previous
"""BioSSMMixer distributed Trainium2 kernel (8 NeuronCores).

Sharding: channel dimension D is split across the 8 cores (the SSM scan is
diagonal in D, so each core scans its own 128 channels with no cross-core
state). The final W_out projection contracts the full D, so the gate tensor
g = y_sp * silu(z) is exchanged with an AllToAll (d-shards -> t-shards) and
each core computes the output rows for its own T/8 slice. The AllToAll and
output matmul for batch row b=0 overlap the b=1 scan.

Host-side prep (not part of HW exec time): W_xd = W_xz[:, :D] @ W_dt is
folded so dt can be computed directly from h (each core only ever needs its
own 128 output channels of x/z/dt); h is pre-transposed to [D, B*T] bf16 so
the contraction axis lands on SBUF partitions without an on-device transpose.

Device layout: all per-channel tensors live as [d=128 partitions, (b,t) free]
tiles. Per (b, n) the recurrence s_t = decay_t*s_{t-1} + inp_t runs as a
single tensor_tensor_scan over the t axis. Bm/Cm rows (which vary with t but
not d) are broadcast across partitions with a one-hot-selector PE matmul.
The y accumulation over n runs on the otherwise-idle GpSimd engine.
"""

import os
import numpy as np
import ml_dtypes

B, T, D, N = 2, 1024, 1024, 16
NCORES = 8
DL = D // NCORES        # 128 channels per core
TL = T // NCORES        # 128 timesteps per core (output slice)
R = B * T               # 2048 rows, b-major: row = b*T + t
KT = D // 128           # 8 contraction tiles
H = 512                 # psum half-tile

BF16 = ml_dtypes.bfloat16

# Filled by kernel() when KERNEL_TRACE=1: exec_time_ns, trace tmpdir.
LAST = {}

_GRAPH_CACHE = {}


def _patch_act_tables():
    """Order activation tables so Exp and Ln resolve to the combined
    natural_log_exp_and_others table (otherwise the table-load pass
    ping-pongs between exp_and_others and natural_log)."""
    import concourse.hw_specs as hw_specs
    import concourse.bacc as bacc_mod
    orig = hw_specs.get_activation_tables.__wrapped__
    import functools

    @functools.cache
    def reordered(arch):
        # Table index (act_func_set_id) must stay aligned with
        # act_info.json's order — never reorder. Prune Exp/Ln from the
        # single-function tables instead so the load pass resolves both
        # to natural_log_exp_and_others.
        import concourse.mybir as mybir
        Act = mybir.ActivationFunctionType
        t = {k: set(v) for k, v in orig(arch).items()}
        if "natural_log_exp_and_others" in t:
            for k in ("exp_and_others", "exp_and_friends"):
                t.get(k, set()).discard(Act.Exp)
            t.get("natural_log", set()).discard(Act.Ln)
        return t

    hw_specs.get_activation_tables = reordered
    bacc_mod.get_activation_tables = reordered


def _build_graph():
    if "nc" in _GRAPH_CACHE:
        return _GRAPH_CACHE["nc"]

    import concourse.bacc as bacc
    import concourse.mybir as mybir
    from concourse import tile

    if os.environ.get('ACT_PATCH','1')=='1':
        _patch_act_tables()

    f32 = mybir.dt.float32
    bf16 = mybir.dt.bfloat16
    Alu = mybir.AluOpType
    Act = mybir.ActivationFunctionType

    nc = bacc.Bacc(
        "TRN2",
        target_bir_lowering=False,
        debug=False,
        enable_asserts=True,
        num_devices=NCORES,
    )

    hT_d = nc.dram_tensor("hT", [B, KT, 128, T], bf16, kind="ExternalInput")
    wpack_d = nc.dram_tensor("wpack", [D, 3 * DL + 2 * N], bf16,
                             kind="ExternalInput")
    wout_d = nc.dram_tensor("wout", [D, D], bf16, kind="ExternalInput")
    acol_d = nc.dram_tensor("acol", [DL, N], f32, kind="ExternalInput")
    bdt_d = nc.dram_tensor("bdt", [DL, 1], f32, kind="ExternalInput")
    dsk_d = nc.dram_tensor("dsk", [DL, 1], f32, kind="ExternalInput")
    nvth_d = nc.dram_tensor("nvth", [DL, 1], f32, kind="ExternalInput")
    hres_d = nc.dram_tensor("hres", [B, TL, D], f32, kind="ExternalInput")
    sel_d = nc.dram_tensor("sel", [2 * N, 2 * N * 128], bf16,
                           kind="ExternalInput")
    out_d = nc.dram_tensor("out", [B, TL, D], f32, kind="ExternalOutput")

    with tile.TileContext(nc) as tc:
        with (
            tc.tile_pool(name="const", bufs=1) as cpool,
            tc.tile_pool(name="work", bufs=1) as wpool,
            tc.tile_pool(name="sc", bufs=4) as scpool,
            tc.tile_pool(name="px", bufs=2, space="PSUM") as pxpool,
            tc.tile_pool(name="dram", bufs=1, space="DRAM") as dpool,
        ):
            # ---- constant loads -------------------------------------------
            # Issue order matters: each queue delivers FIFO, so weights go
            # first (PE's first matmul gates on them), then hT b=0 tiles,
            # then hT b=1 and everything the out stage needs.
            hT = cpool.tile([128, KT, R], bf16)
            _eng = [nc.sync, nc.scalar, nc.gpsimd]
            WP = 3 * DL + 2 * N
            wpk = cpool.tile([128, KT, WP], bf16)
            acol = cpool.tile([DL, N], f32)
            bdt = cpool.tile([DL, 1], f32)
            dsk = cpool.tile([DL, 1], f32)
            nvth = cpool.tile([DL, 1], f32)
            sel = cpool.tile([2 * N, 2 * N * 128], bf16)
            wout = cpool.tile([128, KT, D], bf16)
            hres0 = cpool.tile([TL, D], f32)
            hres1 = cpool.tile([TL, D], f32)
            nc.gpsimd.dma_start(bdt[:], bdt_d[:])
            nc.gpsimd.dma_start(acol[:], acol_d[:])
            nc.gpsimd.dma_start(dsk[:], dsk_d[:])
            nc.gpsimd.dma_start(nvth[:], nvth_d[:])
            # warmup collective: absorbs the one-time CC route setup and the
            # SPMD launch skew (~30-45us) under the input ramp, so the real
            # AllToAlls later run at warm speed (~10us instead of ~40us)
            warm_in = dpool.tile([NCORES, 1], f32, tag="warmi")
            warm_out = dpool.tile([NCORES, 1], f32, tag="warmo")
            nc.sync.dma_start(warm_in[:], bdt[0:NCORES, :])
            nc.gpsimd.collective_compute(
                "AllToAll", Alu.bypass,
                replica_groups=[list(range(NCORES))],
                ins=[warm_in[:].opt()], outs=[warm_out[:].opt()])
            for j in range(KT):
                _eng[j % 3].dma_start(wpk[:, j, :],
                                      wpack_d[j * 128:(j + 1) * 128, :])
            for half in range(B):
                ts_h = slice(half * T, (half + 1) * T)
                for j in range(KT):
                    _eng[j % 3].dma_start(hT[:, j, ts_h], hT_d[half, j])
                if half == 0:
                    nc.sync.dma_start(sel[:], sel_d[:])
            # out-stage tensors ride the scalar HW queue after the hT ramp;
            # they land by ~60us, well before the out matmuls need them
            nc.scalar.dma_start(hres0[:], hres_d[0])
            nc.scalar.dma_start(hres1[:], hres_d[1])
            for j in range(KT):
                nc.scalar.dma_start(wout[:, j, :],
                                    wout_d[j * 128:(j + 1) * 128, :])

            # ---- projections: xT/zT/dtT [128 d, R], BmCm [32, R] ----------
            xT = wpool.tile([128, R], bf16)
            dtT = wpool.tile([128, R], bf16)
            zT = wpool.tile([128, R], bf16)
            dtx = wpool.tile([128, R], bf16)
            yT = wpool.tile([128, R], bf16)
            yT2 = wpool.tile([128, T], bf16)
            bmcm = wpool.tile([2 * N, R], bf16)
            gT = wpool.tile([128, R], bf16)

            for bb in range(B):
                cs = slice(bb * T, (bb + 1) * T)
                pd = pxpool.tile([128, T], f32, tag="pb", bufs=3)
                for hh in range(2):
                    hs = slice(bb * T + hh * H, bb * T + (hh + 1) * H)
                    for j in range(KT):
                        nc.tensor.matmul(pd[:, hh * H:(hh + 1) * H],
                                         wpk[:, j, 2 * DL:3 * DL], hT[:, j, hs],
                                         start=(j == 0), stop=(j == KT - 1))
                # softplus(x+b) = ln(1 + exp(x+b)); Exp and Ln share a table
                et = scpool.tile([128, T], bf16, tag="et", bufs=1)
                nc.scalar.activation(et[:], pd[:], Act.Exp, bias=bdt[:, 0:1])
                nc.scalar.activation(dtT[:, cs], et[:], Act.Ln, bias=1.0)
                px = pxpool.tile([128, T], f32, tag="pb", bufs=3)
                for hh in range(2):
                    hs = slice(bb * T + hh * H, bb * T + (hh + 1) * H)
                    for j in range(KT):
                        nc.tensor.matmul(px[:, hh * H:(hh + 1) * H],
                                         wpk[:, j, 0:DL], hT[:, j, hs],
                                         start=(j == 0), stop=(j == KT - 1))
                nc.vector.tensor_copy(xT[:, cs], px[:])
                pm = pxpool.tile([32, T], f32, tag="pb", bufs=3)
                for hh in range(2):
                    hs = slice(bb * T + hh * H, bb * T + (hh + 1) * H)
                    for j in range(KT):
                        nc.tensor.matmul(pm[:, hh * H:(hh + 1) * H],
                                         wpk[:, j, 3 * DL:WP], hT[:, j, hs],
                                         start=(j == 0), stop=(j == KT - 1))
                nc.vector.tensor_copy(bmcm[:, cs], pm[:])
                nc.vector.tensor_mul(dtx[:, cs], dtT[:, cs], xT[:, cs])

            # ---- per-b: scan over (n), epilogue, AllToAll, out matmul -----
            gT_r = gT[:].rearrange("p (b t) -> p b t", b=B)

            def z_proj(b):
                # z projection is only needed by b's epilogue; emitted late
                # so PE runs it after the time-critical broadcasts
                pz = pxpool.tile([128, T], f32, tag="pb", bufs=3)
                for hh in range(2):
                    hs = slice(b * T + hh * H, b * T + (hh + 1) * H)
                    for j in range(KT):
                        nc.tensor.matmul(pz[:, hh * H:(hh + 1) * H],
                                         wpk[:, j, DL:2 * DL], hT[:, j, hs],
                                         start=(j == 0), stop=(j == KT - 1))
                nc.vector.tensor_copy(zT[:, b * T:(b + 1) * T], pz[:])

            def n_loop(b):
                bs = slice(b * T, (b + 1) * T)
                for n in range(N):
                    decay = scpool.tile([128, T], bf16, tag="decay", bufs=3)
                    nc.scalar.activation(decay[:], dtT[:, bs], Act.Exp,
                                         scale=acol[:, n:n + 1])
                    # Bm broadcast: PE selector matmul into PSUM, then the
                    # scalar engine copies it out as bf16; the inp multiply
                    # then runs in DVE 2x mode (all-bf16 SBUF operands).
                    # Scalar stays a pure producer queue so it pipelines;
                    # gpsimd keeps only end-of-chain consumers (y-accs)
                    pbm = pxpool.tile([128, T], f32, tag="pb", bufs=3)
                    for hh in range(2):
                        hs_d = slice(hh * H, (hh + 1) * H)
                        hs_s = slice(b * T + hh * H, b * T + (hh + 1) * H)
                        nc.tensor.matmul(pbm[:, hs_d],
                                         sel[:, n * 128:(n + 1) * 128],
                                         bmcm[:, hs_s], start=True, stop=True)
                    bmb = scpool.tile([128, T], bf16, tag="bmb", bufs=3)
                    nc.scalar.activation(bmb[:], pbm[:], Act.Copy)
                    inp = scpool.tile([128, T], bf16, tag="inp", bufs=3)
                    nc.vector.tensor_mul(inp[:], dtx[:, bs], bmb[:])
                    s = scpool.tile([128, T], bf16, tag="s", bufs=3)
                    nc.vector.tensor_tensor_scan(s[:], decay[:], inp[:], 0.0,
                                                 Alu.mult, Alu.add)
                    # Cm broadcast: PE selector matmul into PSUM, then the
                    # scalar engine copies it out as bf16 so the tmp multiply
                    # also runs at DVE 2x
                    pcm = pxpool.tile([128, T], f32, tag="pb", bufs=3)
                    for hh in range(2):
                        hs_d = slice(hh * H, (hh + 1) * H)
                        hs_s = slice(b * T + hh * H, b * T + (hh + 1) * H)
                        nc.tensor.matmul(pcm[:, hs_d],
                                         sel[:, (N + n) * 128:(N + n + 1) * 128],
                                         bmcm[:, hs_s], start=True, stop=True)
                    cmb = scpool.tile([128, T], bf16, tag="cmb", bufs=3)
                    nc.scalar.activation(cmb[:], pcm[:], Act.Copy)
                    tmp = scpool.tile([128, T], bf16, tag="tmp", bufs=2)
                    nc.vector.tensor_mul(tmp[:], s[:], cmb[:])
                    # two parallel accumulation chains halve the serial
                    # latency of the y-reduction; the last four adds run on
                    # DVE (0.7us each) to shorten the chain into the epilogue
                    acc = yT[:, bs] if n % 2 == 0 else yT2[:]
                    if n < 2:
                        nc.gpsimd.tensor_copy(acc, tmp[:])
                    elif n < N - 4:
                        nc.gpsimd.tensor_add(acc, acc, tmp[:])
                    else:
                        nc.vector.tensor_add(acc, acc, tmp[:])
                    if b == 1 and n == 1:
                        z_proj(1)
                nc.vector.tensor_add(yT[:, bs], yT[:, bs], yT2[:])

            def epilogue(b):
                # y += D_skip*x ; spike = sigmoid(10y - 10vth) ; g = y*spk*silu(z)
                # y-independent part first: silu(z) overlaps the y chain
                bs = slice(b * T, (b + 1) * T)
                sgz = scpool.tile([128, T], bf16, tag="sgz", bufs=1)
                nc.scalar.activation(sgz[:], zT[:, bs], Act.Sigmoid)
                tz = scpool.tile([128, T], bf16, tag="t2", bufs=1)
                nc.vector.tensor_mul(tz[:], sgz[:], zT[:, bs])
                nc.vector.scalar_tensor_tensor(yT[:, bs], xT[:, bs],
                                               dsk[:, 0:1], yT[:, bs],
                                               Alu.mult, Alu.add)
                spk = scpool.tile([128, T], bf16, tag="spk", bufs=1)
                nc.scalar.activation(spk[:], yT[:, bs], Act.Sigmoid,
                                     scale=10.0, bias=nvth[:, 0:1])
                t1 = scpool.tile([128, T], bf16, tag="t1", bufs=1)
                nc.vector.tensor_mul(t1[:], spk[:], tz[:])
                nc.vector.tensor_mul(gT[:, bs], t1[:], yT[:, bs])

            a2a_out_t = [None, None]

            def a2a(b):
                # AllToAll this b's g: d-shards -> t-shards
                a2a_in = dpool.tile([NCORES, DL, TL], bf16, tag=f"a2ai{b}")
                a2a_out = dpool.tile([NCORES, DL, TL], bf16, tag=f"a2ao{b}")
                nc.sync.dma_start(
                    a2a_in[:].rearrange("j p t -> p j t"),
                    gT_r[:, b, :].rearrange("p (j t) -> p j t", j=NCORES))
                nc.gpsimd.collective_compute(
                    "AllToAll",
                    Alu.bypass,
                    replica_groups=[list(range(NCORES))],
                    ins=[a2a_in[:].opt()],
                    outs=[a2a_out[:].opt()],
                )
                a2a_out_t[b] = a2a_out

            def ga_load(b):
                ga = wpool.tile([128, NCORES, TL], bf16, tag=f"ga{b}")
                nc.sync.dma_start(ga[:],
                                  a2a_out_t[b][:].rearrange("j p t -> p j t"))
                return ga

            def out_mm(b, ga):
                # out rows for this b: g_full @ W_out (residual sub deferred)
                osb = wpool.tile([TL, D], f32, tag=f"osb{b}")
                pos = []
                for eh in range(2):
                    es = slice(eh * H, (eh + 1) * H)
                    po = pxpool.tile([128, H], f32, tag="px")
                    for j in range(NCORES):
                        nc.tensor.matmul(po[:], ga[:, j, :], wout[:, j, es],
                                         start=(j == 0), stop=(j == NCORES - 1))
                    pos.append(po)
                return osb, pos

            def out_sub_store(b, osb, pos):
                hres_t = hres0 if b == 0 else hres1
                for eh in range(2):
                    es = slice(eh * H, (eh + 1) * H)
                    nc.vector.tensor_sub(osb[:, es], pos[eh][:], hres_t[:, es])
                    nc.sync.dma_start(out_d[b][:, es], osb[:, es])

            # b=0's out matmuls are deferred past b=1's n-loop so the
            # in-order PE queue never head-of-line blocks on the collective
            n_loop(0)
            z_proj(0)
            epilogue(0)
            a2a(0)
            n_loop(1)           # overlaps b=0's AllToAll
            ga0 = ga_load(0)
            osb0, pos0 = out_mm(0, ga0)
            epilogue(1)
            out_sub_store(0, osb0, pos0)
            a2a(1)
            ga1 = ga_load(1)
            osb1, pos1 = out_mm(1, ga1)
            out_sub_store(1, osb1, pos1)

    nc.compile()
    _GRAPH_CACHE["nc"] = nc
    return nc


def _install_ntff_hook_shim():
    """This image's antenv package lacks axon_hooks; recreate it with the
    ctypes NTFF hook from trn_agent_boot so trace=True yields exec_time_ns."""
    import sys
    import types
    try:
        import antenv.axon_hooks  # noqa: F401
        return
    except ImportError:
        pass
    import antenv
    mod = types.ModuleType("antenv.axon_hooks")
    _h = {"v": None}
    mod.set_axon_ntff_profile_hook = lambda hook: _h.update(v=hook)
    mod.get_axon_ntff_profile_hook = lambda: _h["v"]
    sys.modules["antenv.axon_hooks"] = mod
    antenv.axon_hooks = mod
    try:
        from trn_agent_boot.trn_boot import _ntff_profile_via_ctypes
        hook = _ntff_profile_via_ctypes("/opt/axon/libaxon_pjrt.so")
        mod.set_axon_ntff_profile_hook(hook)
    except Exception as e:  # degrade to no-trace
        print(f"ntff hook shim failed: {e}")


def kernel(hidden_states, W_xz, W_dt, b_dt, A_log, W_B, W_C, D_skip, W_out,
           v_th):
    h = np.asarray(hidden_states, np.float32)
    Wxz = np.asarray(W_xz, np.float32)
    Wdt = np.asarray(W_dt, np.float32)
    bdt = np.asarray(b_dt, np.float32)
    Alog = np.asarray(A_log, np.float32)
    WB = np.asarray(W_B, np.float32)
    WC = np.asarray(W_C, np.float32)
    Dsk = np.asarray(D_skip, np.float32)
    Wout = np.asarray(W_out, np.float32)
    vth = np.asarray(v_th, np.float32)

    # [B, KT, 128, T] so each per-tile DMA reads one contiguous 256KB block
    hT = np.ascontiguousarray(
        h.transpose(2, 0, 1).reshape(KT, 128, B, T).transpose(2, 0, 1, 3)
    ).astype(BF16)
    Wxd = (Wxz[:, :D].astype(np.float64) @ Wdt.astype(np.float64)).astype(
        np.float32)
    A = -np.exp(Alog)
    wbc = np.concatenate([WB, WC], axis=1)
    wout_bf = Wout.astype(BF16)
    sel_np = np.zeros((2 * N, 2 * N * 128), dtype=BF16)
    for n in range(2 * N):
        sel_np[n, n * 128:(n + 1) * 128] = 1.0

    in_maps = []
    for k in range(NCORES):
        ds = slice(k * DL, (k + 1) * DL)
        ts = slice(k * TL, (k + 1) * TL)
        in_maps.append({
            "hT": hT,
            "wpack": np.ascontiguousarray(np.concatenate(
                [Wxz[:, :D][:, ds], Wxz[:, D:][:, ds], Wxd[:, ds], wbc],
                axis=1)).astype(BF16),
            "wout": wout_bf,
            "acol": np.ascontiguousarray(A[ds, :]),
            "bdt": np.ascontiguousarray(bdt[ds].reshape(DL, 1)),
            "dsk": np.ascontiguousarray(Dsk[ds].reshape(DL, 1)),
            "nvth": np.ascontiguousarray(
                (-10.0 * np.maximum(vth[ds], 0.1)).reshape(DL, 1)),
            "hres": np.ascontiguousarray(h[:, ts, :]),
            "sel": sel_np,
        })

    from concourse.bass_utils import run_bass_kernel_spmd

    nc = _build_graph()
    trace = os.environ.get("KERNEL_TRACE", "0") == "1"
    kwargs = {}
    if trace:
        _install_ntff_hook_shim()
        import tempfile
        tmpdir = tempfile.mkdtemp(prefix="biossm_trace_")
        kwargs = dict(trace=True, tmpdir=tmpdir)
        LAST["trace_dir"] = tmpdir
    try:
        res = run_bass_kernel_spmd(nc, in_maps, core_ids=list(range(NCORES)),
                                   **kwargs)
    except Exception:
        # one retry: a crashed prior run can leave sticky device state that
        # clears on the next attempt
        res = run_bass_kernel_spmd(nc, in_maps, core_ids=list(range(NCORES)),
                                   **kwargs)
    LAST["exec_time_ns"] = getattr(res, "exec_time_ns", None)
    out = np.concatenate(
        [np.asarray(res.results[i]["out"], np.float32) for i in range(NCORES)],
        axis=1)
    return out

